# revision 1
# baseline (speedup 1.0000x reference)
"""Self-contained Trainium2 kernel for the moe_routing tree-walk problem.

Problem (hardcoded shapes): x [16384, 1024] f32, keys/values [4095, 8, 1024]
f32. For each sample and each of 8 trees, walk a depth-12 binary tree: at
each level lam = <x, key[node]>, y += lam * value[node],
node = 2*node + 1 + (lam > 0).

Strategy: data-parallel over the batch across 8 NeuronCores (2048 samples
per core), tables replicated per core.

Per 128-sample tile:
- Levels 0..7 ("dense"): lam for ALL nodes of the level is computed on the
  tensor engine as xT-chunk @ transposed-key-slab matmuls (slabs are
  SBUF-resident, ~16 MB total); the per-sample lam is selected with an
  iota/is_equal mask and a fused multiply+row-reduce. The y update uses a
  one-hot-times-lam matrix W^T (built from PE-transposed node/lam rows via a
  tiny expansion matmul, so it lands nodes-on-partitions) and accumulates
  W^T.T @ V_slab into PSUM-resident y. No gathers at all for these levels.
- Levels 8..11 ("deep"): per-(sample,tree) key/value rows fetched with
  single-index indirect gather DMAs; lam via fused multiply+row-reduce on
  DVE; y += diag(lam) @ v_row on the tensor engine.

Values are cast to bf16 on the host (halves value traffic; y error ~1e-3
relative). Keys/x/lam stay f32 so routing matches the reference up to f32
rounding.
"""

import numpy as np
import ml_dtypes

from contextlib import ExitStack

import concourse.bass as bass
import concourse.mybir as mybir
import concourse.tile as tile
from concourse.bass import IndirectOffsetOnAxis
from concourse.bass_utils import run_bass_kernel_spmd
from concourse.masks import make_identity

# ---------------------------------------------------------------------------
# Workaround: this walrus build rejects >1 sync wait on most instructions
# ("Too many sync wait commands"), but the Tile scheduler can attach several.
# Post-pass: move excess waits onto single-wait NoOps inserted just before
# the instruction on the same engine (program order makes this equivalent).
_WSPLIT_COUNT = [0]


def _split_multi_waits(nc, cap=1):
    for fn in nc.m.functions:
        for bb in fn.blocks:
            out = []
            changed = False
            for inst in list(bb.instructions):
                si = inst.sync_info
                if si is not None and si.on_wait and len(si.on_wait) > cap:
                    waits = list(si.on_wait)
                    extra, keep = waits[:-cap], waits[-cap:]
                    for w in extra:
                        _WSPLIT_COUNT[0] += 1
                        nop = mybir.InstNoOp(
                            name=f"WSPLIT-{_WSPLIT_COUNT[0]}", ins=[], outs=[]
                        )
                        nop.engine = inst.engine
                        nop.sync_info = mybir.SyncInfo(on_wait=[w], on_update=[])
                        out.append(nop)
                    inst.sync_info = mybir.SyncInfo(
                        on_wait=keep, on_update=list(si.on_update or [])
                    )
                    changed = True
                out.append(inst)
            if changed:
                bb.instructions = out
# ---------------------------------------------------------------------------

F32 = mybir.dt.float32
BF16 = mybir.dt.bfloat16
I32 = mybir.dt.int32
OP = mybir.AluOpType

N_CORES = 8
B, D, K, DEPTH = 16384, 1024, 8, 12
N_NODES = 2 ** DEPTH - 1  # 4095
BL = B // N_CORES         # 2048 samples per core
DENSE_LEVELS = 8
P = 128


def _pad_layout(dense_levels):
    pad_offs, off = [], 0
    for l in range(dense_levels):
        pad_offs.append(off)
        off += ((K * 2 ** l + P - 1) // P) * P
    return pad_offs, off


def host_prep(x_shard, keys, values, dense_levels=DENSE_LEVELS):
    """keys/values: [n_nodes, K, D] f32 arrays. Returns per-core input dict
    pieces shared across cores (slabs) and the x-derived arrays."""
    Bl, Dd = x_shard.shape
    T = Bl // P
    DC = Dd // P
    L = dense_levels

    xT4 = np.ascontiguousarray(
        x_shard.reshape(T, P, DC, P).transpose(3, 0, 2, 1)
    )

    ksecs = []
    for l in range(L):
        base, N_l = 2 ** l - 1, 2 ** l
        kl = keys[base:base + N_l]
        klT = kl.transpose(2, 1, 0).reshape(DC, P, K * N_l)
        ksecs.append(klT)
    kTs = np.ascontiguousarray(np.concatenate(ksecs, axis=2).transpose(1, 0, 2))

    vsecs, esecs = [], []
    pad_offs, CSHP = _pad_layout(L)
    for l in range(L):
        base, N_l = 2 ** l - 1, 2 ** l
        rows = K * N_l
        prows = ((rows + P - 1) // P) * P
        vl = np.zeros((prows, Dd), dtype=np.float32)
        vl[:rows] = values[base:base + N_l].transpose(1, 0, 2).reshape(rows, Dd)
        vsecs.append(vl)
        el = np.zeros((9, prows), dtype=np.float32)
        for t in range(K):
            el[t, t * N_l:(t + 1) * N_l] = 1.0
        el[8, :rows] = -np.tile(np.arange(N_l, dtype=np.float32), K)
        el[8, rows:] = 1e9
        esecs.append(el)
    vsh = np.ascontiguousarray(np.concatenate(vsecs, axis=0)).astype(
        ml_dtypes.bfloat16)
    expand = np.ascontiguousarray(np.concatenate(esecs, axis=1))
    return xT4, kTs, vsh, expand, CSHP


def build_kernel(nc, *, Bl=BL, Dd=D, depth=DEPTH, n_nodes=N_NODES,
                 dense_levels=DENSE_LEVELS, repeat=1):
    NK = n_nodes * K
    T = Bl // P
    DC = Dd // P
    L = dense_levels
    CK = K * (2 ** L - 1)
    pad_offs, CSHP = _pad_layout(L)
    NCH = CSHP // P

    koffs = [K * (2 ** l - 1) for l in range(L + 1)]

    x_d = nc.declare_dram_parameter("x", [Bl, Dd], F32, isOutput=False)
    xT_d = nc.declare_dram_parameter("xT4", [P, T, DC, P], F32, isOutput=False)
    kTs_d = nc.declare_dram_parameter("kTs", [P, DC, CK], F32, isOutput=False)
    vsh_d = nc.declare_dram_parameter("vsh", [CSHP, Dd], BF16, isOutput=False)
    exp_d = nc.declare_dram_parameter("expand", [9, CSHP], F32, isOutput=False)
    KVW = Dd + Dd // 2   # f32 key row + bf16 value row viewed as f32
    kv_d = nc.declare_dram_parameter("kv", [NK, KVW], F32, isOutput=False)
    y_d = nc.declare_dram_parameter("y", [Bl, Dd], F32, isOutput=True)

    n_half = (Dd + 511) // 512
    NH = Dd // n_half
    max_Nl = 2 ** (L - 1)

    with ExitStack() as ctx:
        tc = ctx.enter_context(tile.TileContext(nc))
        const_p = ctx.enter_context(tc.tile_pool(name="const", bufs=1))
        xp = ctx.enter_context(tc.tile_pool(name="x", bufs=2))
        gp = ctx.enter_context(tc.tile_pool(name="gather", bufs=5))
        sp = ctx.enter_context(tc.tile_pool(name="small", bufs=4))
        scr = ctx.enter_context(tc.tile_pool(name="scratch", bufs=3))
        wp = ctx.enter_context(tc.tile_pool(name="wtile", bufs=4))
        yp = ctx.enter_context(tc.tile_pool(name="yout", bufs=2))
        psy = ctx.enter_context(tc.tile_pool(name="psy", bufs=1, space="PSUM"))
        psl = ctx.enter_context(tc.tile_pool(name="psl", bufs=2, space="PSUM"))
        psb = ctx.enter_context(tc.tile_pool(name="psb", bufs=2, space="PSUM"))

        kTs_sb = const_p.tile([P, DC, CK], F32)
        nc.sync.dma_start(kTs_sb[:], kTs_d[:])
        vsh_sb = const_p.tile([P, NCH, Dd], BF16)
        nc.sync.dma_start(vsh_sb[:], vsh_d[:].rearrange("(q p) d -> p q d", p=P))
        exp_sb = const_p.tile([9, CSHP], F32)
        nc.sync.dma_start(exp_sb[:], exp_d[:])

        ident_b = const_p.tile([P, P], BF16)
        make_identity(nc, ident_b[:])
        ident_f = const_p.tile([P, P], F32)
        make_identity(nc, ident_f[:])
        tree_off = const_p.tile([P, K], I32)
        nc.gpsimd.iota(tree_off[:], pattern=[[1, K]], base=0,
                       channel_multiplier=0)
        iota_row_i = const_p.tile([P, max_Nl], I32)
        nc.gpsimd.iota(iota_row_i[:], pattern=[[1, max_Nl]], base=0,
                       channel_multiplier=0)
        iota_row = const_p.tile([P, max_Nl], F32)
        nc.vector.tensor_copy(iota_row[:], iota_row_i[:])
        rowsN = const_p.tile([9, P], F32)
        nc.vector.memset(rowsN[:], 1.0)   # row 8 stays 1.0 (ones row)
        rowsL = const_p.tile([9, P], F32)
        nc.vector.memset(rowsL[:], 0.0)   # row 8 stays 0.0

        for rep in range(repeat):
            for t in range(T):
                x_tile = xp.tile([P, Dd], F32, tag="xt")
                nc.sync.dma_start(x_tile[:], x_d[t * P:(t + 1) * P, :])
                xT_tile = xp.tile([P, DC, P], F32, tag="xT")
                nc.sync.dma_start(xT_tile[:], xT_d[:, t, :, :])

                node = sp.tile([P, K], F32, tag="node")  # level-relative index
                nc.vector.memset(node[:], 0.0)

                y_ps = [
                    psy.tile([P, NH], F32, tag=f"yps{h}",
                             name=f"yps{h}_{rep}_{t}")
                    for h in range(n_half)
                ]
                mm_first = [True]

                def y_accum(lhsT, rhs_full, last):
                    for h in range(n_half):
                        nc.tensor.matmul(
                            out=y_ps[h][:],
                            lhsT=lhsT,
                            rhs=rhs_full[:, h * NH:(h + 1) * NH],
                            start=mm_first[0],
                            stop=last,
                        )
                    mm_first[0] = False

                for l in range(depth):
                    N_l = 2 ** l
                    lam = sp.tile([P, K], F32, tag="lam")

                    if l < L:
                        CL = K * N_l
                        lam_ps = psl.tile([P, CL], F32, tag="lamall",
                                          name=f"lamall_{rep}_{t}_{l}")
                        for blk in range(0, CL, 512):
                            bw = min(512, CL - blk)
                            for c in range(DC):
                                nc.tensor.matmul(
                                    out=lam_ps[:, blk:blk + bw],
                                    lhsT=xT_tile[:, c, :],
                                    rhs=kTs_sb[:, c,
                                               koffs[l] + blk:koffs[l] + blk + bw],
                                    start=(c == 0),
                                    stop=(c == DC - 1),
                                )
                        junk2 = scr.tile([P, max_Nl], F32, tag="junk2")
                        for k in range(K):
                            nc.vector.scalar_tensor_tensor(
                                out=junk2[:, :N_l],
                                in0=iota_row[:, :N_l],
                                scalar=node[:, k:k + 1],
                                in1=lam_ps[:, k * N_l:(k + 1) * N_l],
                                op0=OP.is_equal,
                                op1=OP.mult,
                                accum_out=lam[:, k:k + 1],
                            )

                        tr_ps = psb.tile([P, 512], F32, tag="trbc",
                                         name=f"trbc_{rep}_{t}_{l}")
                        nc.tensor.transpose(tr_ps[0:K, 256:256 + P], node[:],
                                            ident_f[:])
                        nc.tensor.transpose(tr_ps[0:K, 256 + P:256 + 2 * P],
                                            lam[:], ident_f[:])
                        nc.scalar.activation(
                            rowsN[0:K, :], tr_ps[0:K, 256:256 + P],
                            mybir.ActivationFunctionType.Copy)
                        nc.scalar.activation(
                            rowsL[0:K, :], tr_ps[0:K, 256 + P:256 + 2 * P],
                            mybir.ActivationFunctionType.Copy)

                        n_chunks = (K * N_l + P - 1) // P
                        last_lvl = (l == depth - 1)
                        for q in range(n_chunks):
                            ecols = exp_sb[:, pad_offs[l] + q * P:
                                           pad_offs[l] + (q + 1) * P]
                            bc_ps = psb.tile([P, 512], F32, tag="trbc",
                                             name=f"bcps_{rep}_{t}_{l}_{q}")
                            nc.tensor.matmul(out=bc_ps[:, 0:P], lhsT=ecols,
                                             rhs=rowsN[:], start=True,
                                             stop=True)
                            nc.tensor.matmul(out=bc_ps[:, P:256], lhsT=ecols,
                                             rhs=rowsL[:], start=True,
                                             stop=True)
                            tmp = scr.tile([P, P], F32, tag="wtmp")
                            nc.vector.tensor_scalar(
                                out=tmp[:], in0=bc_ps[:, 0:P], scalar1=0.0,
                                scalar2=None, op0=OP.is_equal,
                            )
                            W = wp.tile([P, P], BF16, tag="W")
                            nc.vector.tensor_tensor(
                                out=W[:], in0=tmp[:], in1=bc_ps[:, P:256],
                                op=OP.mult,
                            )
                            y_accum(W[:], vsh_sb[:, pad_offs[l] // P + q, :],
                                    last_lvl and q == n_chunks - 1)
                    else:
                        base = N_l - 1
                        nodeg = sp.tile([P, K], F32, tag="nodeg")
                        nc.vector.tensor_scalar(
                            out=nodeg[:], in0=node[:], scalar1=float(K),
                            scalar2=float(base * K), op0=OP.mult, op1=OP.add,
                        )
                        idx = sp.tile([P, K], I32, tag="idx")
                        nc.vector.tensor_copy(idx[:], nodeg[:])
                        nc.vector.tensor_tensor(
                            out=idx[:], in0=idx[:], in1=tree_off[:], op=OP.add
                        )
                        last_lvl = (l == depth - 1)
                        for k in range(K):
                            kvg = gp.tile([P, KVW], F32, tag="kvg")
                            nc.gpsimd.indirect_dma_start(
                                out=kvg[:],
                                out_offset=None,
                                in_=kv_d[:],
                                in_offset=IndirectOffsetOnAxis(
                                    ap=idx[:, k:k + 1], axis=0),
                            )
                            junk = scr.tile([P, Dd], F32, tag="junk")
                            nc.vector.scalar_tensor_tensor(
                                out=junk[:],
                                in0=x_tile[:],
                                scalar=1.0,
                                in1=kvg[:, 0:Dd],
                                op0=OP.mult,
                                op1=OP.mult,
                                accum_out=lam[:, k:k + 1],
                            )
                            diag = wp.tile([P, P], BF16, tag="diag")
                            nc.scalar.activation(
                                diag[:], ident_b[:],
                                mybir.ActivationFunctionType.Copy,
                                scale=lam[:, k:k + 1],
                            )
                            vg = kvg[:, Dd:KVW].bitcast(BF16)
                            y_accum(diag[:], vg, last_lvl and k == K - 1)

                    # node = node*2 + (lam > 0)   (relative index, f32 exact)
                    gt = sp.tile([P, K], F32, tag="gt")
                    nc.vector.tensor_scalar(
                        out=gt[:], in0=lam[:], scalar1=0.0, scalar2=None,
                        op0=OP.is_gt,
                    )
                    nc.vector.tensor_scalar(
                        out=node[:], in0=node[:], scalar1=2.0, scalar2=0.0,
                        op0=OP.mult, op1=OP.add,
                    )
                    nc.vector.tensor_tensor(
                        out=node[:], in0=node[:], in1=gt[:], op=OP.add
                    )

                y_sb = yp.tile([P, Dd], F32)
                for h in range(n_half):
                    nc.scalar.activation(
                        y_sb[:, h * NH:(h + 1) * NH], y_ps[h][:],
                        mybir.ActivationFunctionType.Copy)
                nc.sync.dma_start(y_d[t * P:(t + 1) * P, :], y_sb[:])

    return nc


_NC_CACHE = {}


def _get_nc(repeat=1):
    key = ("nc", repeat)
    if key not in _NC_CACHE:
        nc = bass.Bass("TRN2", target_bir_lowering=False, debug=False,
                       num_devices=N_CORES)
        build_kernel(nc, repeat=repeat)
        _split_multi_waits(nc)
        _NC_CACHE[key] = nc
    return _NC_CACHE[key]


def make_kv(keys_flat_f32, values_flat_bf16):
    NK, Dd = keys_flat_f32.shape
    kv = np.empty((NK, Dd + Dd // 2), dtype=np.float32)
    kv[:, :Dd] = keys_flat_f32
    kv[:, Dd:] = values_flat_bf16.view(np.float32)
    return kv


def _prep_inputs(x, keys, values):
    x = np.ascontiguousarray(np.asarray(x, dtype=np.float32))
    keys = np.asarray(keys, dtype=np.float32)
    values = np.asarray(values, dtype=np.float32)
    keys_flat = np.ascontiguousarray(keys.reshape(N_NODES * K, D))
    values_flat = np.ascontiguousarray(values.reshape(N_NODES * K, D)).astype(
        ml_dtypes.bfloat16)
    kv = make_kv(keys_flat, values_flat)

    # table-derived slabs are identical for every core: compute them once
    _, kTs, vsh, expand, _ = host_prep(x[:BL], keys, values)
    in_maps = []
    for c in range(N_CORES):
        x_shard = x[c * BL:(c + 1) * BL]
        T = BL // P
        DC = D // P
        xT4 = np.ascontiguousarray(
            x_shard.reshape(T, P, DC, P).transpose(3, 0, 2, 1))
        in_maps.append({
            "x": x_shard,
            "xT4": xT4,
            "kTs": kTs,
            "vsh": vsh,
            "expand": expand,
            "kv": kv,
        })
    return in_maps


def kernel(x, keys, values):
    nc = _get_nc()
    in_maps = _prep_inputs(x, keys, values)
    res = run_bass_kernel_spmd(nc, in_maps, list(range(N_CORES)))
    y = np.concatenate([res.results[c]["y"] for c in range(N_CORES)], axis=0)
    return y.astype(np.float32)



# revision 23
# speedup vs baseline: 1.0129x; 1.0129x over previous
"""Self-contained Trainium2 kernel for the moe_routing tree-walk problem.

Problem (hardcoded shapes): x [16384, 1024] f32, keys/values [4095, 8, 1024]
f32. For each sample and each of 8 trees, walk a depth-12 binary tree: at
each level lam = <x, key[node]>, y += lam * value[node],
node = 2*node + 1 + (lam > 0).

Strategy: data-parallel over the batch across 8 NeuronCores (2048 samples
per core), tables replicated per core.

Per 128-sample tile:
- Levels 0..7 ("dense"): lam for ALL nodes of the level is computed on the
  tensor engine as xT-chunk @ transposed-key-slab matmuls (the slab is
  SBUF-resident); the four 512-col chunk groups are copied to an SBUF lam
  slab so PSUM recycles fast. Per-sample lam is selected with an
  iota/is_equal fused multiply+row-reduce. The y update uses a
  one-hot-times-lam matrix W^T (built from PE-transposed node/lam rows via a
  tiny f16 expansion matmul) accumulated into PSUM-resident y.
- Levels 8..11 ("deep"): row indices for all 8 trees are folded into the
  16-partition-wrapped int16 layout that the InstDMAGatherAnt ucode expects
  (one DVE mask-multiply + one PE fold matmul + one cast), then batched
  dma_gather instructions fetch 2-tree quarters of combined key|value rows
  (f32 keys + bf16 values for levels 8-10; all-bf16 rows for level 11, whose
  lam no longer affects routing). lam via fused multiply+row-reduce on DVE;
  y += diag(lam) @ v_row on the tensor engine.

Routing dot products stay f32 end to end (a single route flip costs ~0.2
relative error, far over the gate), value-side math is bf16.
"""

import numpy as np
import ml_dtypes

from contextlib import ExitStack

import concourse.bass as bass
import concourse.mybir as mybir
import concourse.tile as tile
from concourse.bass_utils import run_bass_kernel_spmd
from concourse.library_config import all_libraries, standard

# ---------------------------------------------------------------------------
# Workaround: this walrus build rejects >1 sync wait on most instructions
# ("Too many sync wait commands"), but the Tile scheduler can attach several.
# Post-pass: move excess waits onto single-wait NoOps inserted just before
# the instruction on the same engine (program order makes this equivalent).
_WSPLIT_COUNT = [0]


def _split_multi_waits(nc, cap=1):
    for fn in nc.m.functions:
        for bb in fn.blocks:
            out = []
            changed = False
            for inst in list(bb.instructions):
                si = inst.sync_info
                if si is not None and si.on_wait and len(si.on_wait) > cap:
                    waits = list(si.on_wait)
                    extra, keep = waits[:-cap], waits[-cap:]
                    for w in extra:
                        _WSPLIT_COUNT[0] += 1
                        nop = mybir.InstNoOp(
                            name=f"WSPLIT-{_WSPLIT_COUNT[0]}", ins=[], outs=[]
                        )
                        nop.engine = inst.engine
                        nop.sync_info = mybir.SyncInfo(on_wait=[w], on_update=[])
                        out.append(nop)
                    inst.sync_info = mybir.SyncInfo(
                        on_wait=keep, on_update=list(si.on_update or [])
                    )
                    changed = True
                out.append(inst)
            if changed:
                bb.instructions = out


def _insert_library_loads(nc):
    """Raw-Bass equivalent of Bacc.compile's library handling: place
    LOAD_LIB for the gpsimd ucode library (dma_gather lives in `mlp`) and
    populate .instr bytes for extended-inst ISA subclasses."""
    inst_type_to_lib_mask = {}
    for lib in all_libraries:
        for it in lib.instructions:
            inst_type_to_lib_mask[it] = inst_type_to_lib_mask.get(it, 0) | (
                1 << lib.index
            )
    mybir._bass_rust.insert_library_loads(
        nc, inst_type_to_lib_mask, len(all_libraries), standard.index
    )
    mybir.codegen_inst_isa_subclasses(nc)
# ---------------------------------------------------------------------------

F32 = mybir.dt.float32
F16 = mybir.dt.float16
BF16 = mybir.dt.bfloat16
I16 = mybir.dt.int16
OP = mybir.AluOpType

N_CORES = 8
B, D, K, DEPTH = 16384, 1024, 8, 12
N_NODES = 2 ** DEPTH - 1  # 4095
NK = N_NODES * K          # 32760 rows; fits int16 (max row id 32759)
BL = B // N_CORES         # 2048 samples per core
DENSE_LEVELS = 8
P = 128
KVW = D + D // 2          # f32 key row + bf16 value row viewed as f32
KVBW = 2 * D              # bf16 key row + bf16 value row (level 11)


def _pad_layout(dense_levels):
    """exp-matrix column offsets: one 128-wide section per (level, chunk)."""
    pad_offs, off = [], 0
    for l in range(dense_levels):
        pad_offs.append(off)
        off += ((K * 2 ** l + P - 1) // P) * P
    return pad_offs, off


def _packed_layout(dense_levels):
    """vsh row offsets: small levels share 128-row chunks."""
    offs, off = [], 0
    for l in range(dense_levels):
        rows = K * 2 ** l
        if off % P + rows > P and off % P != 0:
            off += P - off % P
        offs.append(off)
        off += rows
    total = ((off + P - 1) // P) * P
    return offs, total


def host_prep(keys, values, dense_levels=DENSE_LEVELS):
    """keys/values: [n_nodes, K, D] f32. Returns table-derived slabs shared
    across cores."""
    Dd = D
    DC = Dd // P
    L = dense_levels

    ksecs = []
    for l in range(L):
        base, N_l = 2 ** l - 1, 2 ** l
        kl = keys[base:base + N_l]
        klT = kl.transpose(2, 1, 0).reshape(DC, P, K * N_l)
        ksecs.append(klT)
    kTs = np.ascontiguousarray(np.concatenate(ksecs, axis=2).transpose(1, 0, 2))

    pad_offs, ECOLS = _pad_layout(L)
    voffs, VSHP = _packed_layout(L)
    vsh = np.zeros((VSHP, Dd), dtype=np.float32)
    expand = np.zeros((9, ECOLS), dtype=np.float32)
    for l in range(L):
        base, N_l = 2 ** l - 1, 2 ** l
        rows = K * N_l
        vsh[voffs[l]:voffs[l] + rows] = (
            values[base:base + N_l].transpose(1, 0, 2).reshape(rows, Dd))
        # expand section for level l: n_chunks 128-col blocks addressing the
        # packed vsh chunks that hold this level's rows
        inchunk = voffs[l] % P
        n_chunks = (inchunk + rows + P - 1) // P
        el = np.zeros((9, n_chunks * P), dtype=np.float32)
        el[8, :] = 60000.0
        for t in range(K):
            el[t, inchunk + t * N_l: inchunk + (t + 1) * N_l] = 1.0
        el[8, inchunk: inchunk + rows] = -np.tile(
            np.arange(N_l, dtype=np.float32), K)
        expand[:, pad_offs[l]:pad_offs[l] + n_chunks * P] = el
    vsh = vsh.astype(ml_dtypes.bfloat16)
    # integers <= 2048 are exact in f16; bias placeholder 60000 < f16 max
    expand = expand.astype(np.float16)
    return kTs, vsh, expand, VSHP


def host_consts():
    """Constant tiles (identity/iota/fold/mask) shipped from the host so the
    gpsimd engine never needs the `standard` ucode library."""
    identf = np.eye(P, dtype=np.float32)
    identb = np.eye(P, dtype=np.float32).astype(ml_dtypes.bfloat16)
    iota_row = np.tile(np.arange(P, dtype=np.float32), (P, 1))
    foldrep = np.zeros((P, P), dtype=np.float32)
    for p in range(P):
        for q in range(P):
            if p % 16 == q % 16:
                foldrep[p, q] = 1.0
    mask64 = np.zeros((P, K, 8), dtype=np.float32)
    for p in range(P):
        mask64[p, :, p // 16] = float(K)      # folds the *K into the mask
    ckm = np.zeros((4, P, K, 8), dtype=np.float32)
    for li, l in enumerate(range(DENSE_LEVELS, DEPTH)):
        base = 2 ** l - 1
        for p in range(P):
            for k in range(K):
                ckm[li, p, k, p // 16] = float(base * K + k)
    return identf, identb, iota_row, foldrep, mask64, ckm


def build_kernel(nc, *, Bl=BL, Dd=D, depth=DEPTH,
                 dense_levels=DENSE_LEVELS, repeat=1):
    T = Bl // P
    DC = Dd // P
    L = dense_levels
    CK = K * (2 ** L - 1)          # 2040 dense lam columns
    pad_offs, CSHP = _pad_layout(L)

    voffs, VSHP = _packed_layout(L)
    koffs = [K * (2 ** l - 1) for l in range(L + 1)]
    # dense lam chunk groups (start, width): levels 0-5 share one chunk
    groups = [(0, koffs[6]), (koffs[6], 512), (koffs[7], 512), (koffs[7] + 512, 512)]

    x_d = nc.declare_dram_parameter("x", [Bl, Dd], F32, isOutput=False)
    xT_d = nc.declare_dram_parameter("xT4", [P, T, DC, P], F32, isOutput=False)
    kTs_d = nc.declare_dram_parameter("kTs", [P, DC, CK], F32, isOutput=False)
    vsh_d = nc.declare_dram_parameter("vsh", [VSHP, Dd], BF16, isOutput=False)
    exp_d = nc.declare_dram_parameter("expand", [9, CSHP], F16, isOutput=False)
    kv_d = nc.declare_dram_parameter("kv", [NK, KVW], F32, isOutput=False)
    kvb_d = nc.declare_dram_parameter("kvb", [NK, KVBW], BF16, isOutput=False)
    identf_d = nc.declare_dram_parameter("identf", [P, P], F32, isOutput=False)
    identb_d = nc.declare_dram_parameter("identb", [P, P], BF16, isOutput=False)
    iota_d = nc.declare_dram_parameter("iota_row", [P, P], F32, isOutput=False)
    fold_d = nc.declare_dram_parameter("foldrep", [P, P], F32, isOutput=False)
    mask_d = nc.declare_dram_parameter("mask64", [P, K, 8], F32, isOutput=False)
    ckm_d = nc.declare_dram_parameter("ckm", [4, P, K, 8], F32, isOutput=False)
    y_d = nc.declare_dram_parameter("y", [Bl, Dd], F32, isOutput=True)

    NH = 512                        # one PSUM bank of f32
    n_half = Dd // NH

    with ExitStack() as ctx:
        tc = ctx.enter_context(tile.TileContext(nc))
        const_p = ctx.enter_context(tc.tile_pool(name="const", bufs=1))
        xp = ctx.enter_context(tc.tile_pool(name="x", bufs=4))
        xtp = ctx.enter_context(tc.tile_pool(name="xT", bufs=2))
        kvp = ctx.enter_context(tc.tile_pool(name="kvg", bufs=3))
        kvbp = ctx.enter_context(tc.tile_pool(name="kvbg", bufs=2))
        lp = ctx.enter_context(tc.tile_pool(name="lamsb", bufs=1))
        sp = ctx.enter_context(tc.tile_pool(name="small", bufs=8))
        ip = ctx.enter_context(tc.tile_pool(name="idx", bufs=4))
        scr = ctx.enter_context(tc.tile_pool(name="scratch", bufs=2))
        wp = ctx.enter_context(tc.tile_pool(name="wtile", bufs=4))
        lamp = ctx.enter_context(tc.tile_pool(name="lamt", bufs=52))
        nsvp = ctx.enter_context(tc.tile_pool(name="nsv", bufs=40))
        yp = ctx.enter_context(tc.tile_pool(name="yout", bufs=1))
        psy = ctx.enter_context(tc.tile_pool(name="psy", bufs=2, space="PSUM"))
        psl = ctx.enter_context(tc.tile_pool(name="psl", bufs=2, space="PSUM"))
        psb = ctx.enter_context(tc.tile_pool(name="psb", bufs=2, space="PSUM"))

        kTs_sb = const_p.tile([P, DC, CK], F32)
        nc.sync.dma_start(kTs_sb[:], kTs_d[:])
        vsh_sb = const_p.tile([P, VSHP // P, Dd], BF16)
        nc.sync.dma_start(vsh_sb[:], vsh_d[:].rearrange("(q p) d -> p q d", p=P))
        exp_sb = const_p.tile([9, CSHP], F16)
        nc.sync.dma_start(exp_sb[:], exp_d[:])
        ident_f = const_p.tile([P, P], F32)
        nc.sync.dma_start(ident_f[:], identf_d[:])
        ident_b = const_p.tile([P, P], BF16)
        nc.sync.dma_start(ident_b[:], identb_d[:])
        iota_row = const_p.tile([P, P], F32)
        nc.sync.dma_start(iota_row[:], iota_d[:])
        foldrep = const_p.tile([P, P], F32)
        nc.sync.dma_start(foldrep[:], fold_d[:])
        mask64 = const_p.tile([P, K, 8], F32)
        nc.sync.dma_start(mask64[:], mask_d[:])
        ckm = const_p.tile([P, 4, K, 8], F32)
        nc.sync.dma_start(ckm[:], ckm_d[:].rearrange("l p k e -> p l k e"))
        rowsNs, rowsLs = [], []
        for l in range(L):
            rn = const_p.tile([9, P], F16, tag=f"rowsN{l}", name=f"rowsN_{l}")
            nc.vector.memset(rn[:], 1.0)   # row 8 stays 1.0 (ones row)
            rl = const_p.tile([9, P], F16, tag=f"rowsL{l}", name=f"rowsL_{l}")
            nc.vector.memset(rl[:], 0.0)   # row 8 stays 0.0
            rowsNs.append(rn)
            rowsLs.append(rl)

        # one shared register for every dma_gather's num_idxs
        nidx_reg = nc.gpsimd.to_reg(256)

        for rep in range(repeat):
            state = {}

            def emit_A(t):
                """Per-tile prep: x loads, dense lam matmuls, lam slab."""
                st = {}
                st["x"] = xp.tile([P, Dd], F32, tag="xt", name=f"x_{rep}_{t}")
                nc.sync.dma_start(st["x"][:], x_d[t * P:(t + 1) * P, :])
                xT_tile = xtp.tile([P, DC, P], F32, tag="xT",
                                   name=f"xT_{rep}_{t}")
                nc.sync.dma_start(xT_tile[:], xT_d[:, t, :, :])
                st["node"] = sp.tile([P, K], F32, tag="node",
                                     name=f"node_{rep}_{t}")
                nc.vector.memset(st["node"][:], 0.0)
                st["lam_sb"] = lp.tile([P, CK + 8], F32, tag="lamsb",
                                       name=f"lamsb_{rep}_{t}")
                for gi, (goff, gw) in enumerate(groups):
                    lam_ps = psl.tile([P, 512], F32, tag="lamg",
                                      name=f"lamg_{rep}_{t}_{gi}")
                    for c in range(DC):
                        nc.tensor.matmul(
                            out=lam_ps[:, 0:gw],
                            lhsT=xT_tile[:, c, :],
                            rhs=kTs_sb[:, c, goff:goff + gw],
                            start=(c == 0),
                            stop=(c == DC - 1),
                        )
                    nc.scalar.activation(
                        st["lam_sb"][:, goff:goff + gw], lam_ps[:, 0:gw],
                        mybir.ActivationFunctionType.Copy)
                st["y_ps"] = [
                    psy.tile([P, NH], F32, tag=f"yps{h}",
                             name=f"yps{h}_{rep}_{t}")
                    for h in range(n_half)
                ]
                st["mm_first"] = True
                st["nsv"] = {}
                st["lsv"] = {}
                state[t] = st

            def y_accum(st, lhsT, rhs_full, last):
                for h in range(n_half):
                    nc.tensor.matmul(
                        out=st["y_ps"][h][:],
                        lhsT=lhsT,
                        rhs=rhs_full[:, h * NH:(h + 1) * NH],
                        start=st["mm_first"],
                        stop=last,
                    )
                st["mm_first"] = False

            def node_update(st, lam):
                gt = sp.tile([P, K], F32, tag="gt")
                nc.vector.tensor_scalar(
                    out=gt[:], in0=lam[:], scalar1=0.0, scalar2=None,
                    op0=OP.is_gt,
                )
                nc.vector.tensor_scalar(
                    out=st["node"][:], in0=st["node"][:], scalar1=2.0,
                    scalar2=0.0, op0=OP.mult, op1=OP.add,
                )
                nc.vector.tensor_tensor(
                    out=st["node"][:], in0=st["node"][:], in1=gt[:], op=OP.add
                )

            def route_dense_level(t, l):
                """DVE-only routing step: select lam, snapshot, update node."""
                st = state[t]
                node, lam_sb = st["node"], st["lam_sb"]
                N_l = 2 ** l
                lam = lamp.tile([P, K], F32, tag="lam")
                junk2 = scr.tile([P, P], F32, tag="junk2")
                for k in range(K):
                    nc.vector.scalar_tensor_tensor(
                        out=junk2[:, :N_l],
                        in0=iota_row[:, :N_l],
                        scalar=node[:, k:k + 1],
                        in1=lam_sb[:, koffs[l] + k * N_l:
                                   koffs[l] + (k + 1) * N_l],
                        op0=OP.is_equal,
                        op1=OP.mult,
                        accum_out=lam[:, k:k + 1],
                    )
                nsv = nsvp.tile([P, K], F32, tag="nsv")
                nc.vector.tensor_copy(nsv[:], node[:])
                st["nsv"][l] = nsv
                st["lsv"][l] = lam
                node_update(st, lam)

            def fill_dense_level(t, l, last=False):
                """Decoupled y-update for a dense level from saved node/lam."""
                st = state[t]
                node, lam = st["nsv"].pop(l), st["lsv"].pop(l)
                N_l = 2 ** l
                tr_ps = psb.tile([P, 512], F32, tag="trbc",
                                 name=f"trbc_{rep}_{t}_{l}")
                nc.tensor.transpose(tr_ps[0:K, 256:256 + P], node[:],
                                    ident_f[:])
                nc.tensor.transpose(tr_ps[0:K, 256 + P:256 + 2 * P],
                                    lam[:], ident_f[:])
                rowsN, rowsL = rowsNs[l], rowsLs[l]
                nc.scalar.activation(
                    rowsN[0:K, :], tr_ps[0:K, 256:256 + P],
                    mybir.ActivationFunctionType.Copy)
                nc.scalar.activation(
                    rowsL[0:K, :], tr_ps[0:K, 256 + P:256 + 2 * P],
                    mybir.ActivationFunctionType.Copy)

                n_chunks = (voffs[l] % P + K * N_l + P - 1) // P
                for q in range(n_chunks):
                    ecols = exp_sb[:, pad_offs[l] + q * P:
                                   pad_offs[l] + (q + 1) * P]
                    bc_ps = psb.tile([P, 512], F32, tag="trbc",
                                     name=f"bcps_{rep}_{t}_{l}_{q}")
                    nc.tensor.matmul(out=bc_ps[:, 0:P], lhsT=ecols,
                                     rhs=rowsN[0:9, :], start=True, stop=True)
                    nc.tensor.matmul(out=bc_ps[:, P:256], lhsT=ecols,
                                     rhs=rowsL[0:9, :], start=True, stop=True)
                    tmp = scr.tile([P, P], F32, tag="wtmp")
                    nc.vector.tensor_scalar(
                        out=tmp[:], in0=bc_ps[:, 0:P], scalar1=0.0,
                        scalar2=None, op0=OP.is_equal,
                    )
                    W = wp.tile([P, P], BF16, tag="W")
                    nc.vector.tensor_tensor(
                        out=W[:], in0=tmp[:], in1=bc_ps[:, P:256],
                        op=OP.mult,
                    )
                    y_accum(st, W[:], vsh_sb[:, voffs[l] // P + q, :],
                            last and q == n_chunks - 1)

            def deep_level(t, l, last_mm):
                """Deep level: idx fold, batched gathers, dots, y update."""
                st = state[t]
                node, x_tile = st["node"], st["x"]
                li = l - L
                last_lvl = (l == depth - 1)
                lam = lamp.tile([P, K], F32, tag="lam")
                idx64 = ip.tile([P, K, 8], F32, tag="idx64")
                nc.vector.tensor_tensor(
                    out=idx64[:],
                    in0=node[:].rearrange("p (k o) -> p k o", o=1)
                               .broadcast_to([P, K, 8]),
                    in1=mask64[:], op=OP.mult)
                nc.vector.tensor_tensor(
                    out=idx64[:], in0=idx64[:], in1=ckm[:, li, :, :],
                    op=OP.add)
                fold_ps = psb.tile([P, 512], F32, tag="trbc",
                                   name=f"fold_{rep}_{t}_{l}")
                nc.tensor.matmul(
                    out=fold_ps[:, 0:64], lhsT=foldrep[:],
                    rhs=idx64[:].rearrange("p a b -> p (a b)"),
                    start=True, stop=True)
                idx16 = ip.tile([P, 64], I16, tag="idx16")
                nc.vector.tensor_copy(idx16[:], fold_ps[:, 0:64])

                for quar in range(4):
                    iap = idx16[:, 16 * quar:16 * quar + 16]
                    qn = quar % 2
                    if l < depth - 1:
                        kvq = kvp.tile([P, 2, KVW], F32, tag="kvq")
                        nc.gpsimd.dma_gather(
                            kvq[:], kv_d[:], iap, 256, nidx_reg, KVW,
                            queue_num=qn)
                        kpart = [kvq[:, j, 0:Dd] for j in range(2)]
                        vpart = [
                            kvq[:, j, Dd:KVW].bitcast(BF16)
                            for j in range(2)
                        ]
                    else:
                        kvq = kvbp.tile([P, 2, KVBW], BF16, tag="kvbq")
                        nc.gpsimd.dma_gather(
                            kvq[:], kvb_d[:], iap, 256, nidx_reg,
                            KVBW, queue_num=qn)
                        kpart = [kvq[:, j, 0:Dd] for j in range(2)]
                        vpart = [kvq[:, j, Dd:KVBW] for j in range(2)]

                    for j in range(2):
                        k = 2 * quar + j
                        nc.vector.scalar_tensor_tensor(
                            out=kpart[j],
                            in0=x_tile[:],
                            scalar=1.0,
                            in1=kpart[j],
                            op0=OP.mult,
                            op1=OP.mult,
                            accum_out=lam[:, k:k + 1],
                        )
                        diag = wp.tile([P, P], BF16, tag="diag")
                        nc.scalar.activation(
                            diag[:], ident_b[:],
                            mybir.ActivationFunctionType.Copy,
                            scale=lam[:, k:k + 1],
                        )
                        y_accum(st, diag[:], vpart[j],
                                last_mm and last_lvl and k == K - 1)
                if not last_lvl:
                    node_update(st, lam)

            def emit_finish(t):
                st = state[t]
                y_sb = yp.tile([P, Dd], F32, tag="ysb")
                for h in range(n_half):
                    nc.scalar.activation(
                        y_sb[:, h * NH:(h + 1) * NH], st["y_ps"][h][:],
                        mybir.ActivationFunctionType.Copy)
                nc.sync.dma_start(y_d[t * P:(t + 1) * P, :], y_sb[:])
                del state[t]

            # Emission plan: tiles processed in pairs; within a pair the
            # two tiles' deep levels are braided so one tile's gathers fill
            # the other's dependency bubbles. Routing (selects/node updates)
            # is decoupled from all y-update work, which braids in as filler.
            def route_all(t):
                emit_A(t)
                for l in range(L):
                    route_dense_level(t, l)

            route_all(0)
            route_all(1)
            for pr in range(T // 2):
                t0, t1 = 2 * pr, 2 * pr + 1
                if t1 + 2 < T:
                    route_all(t0 + 2)
                    route_all(t1 + 2)
                for li in range(depth - L):
                    deep_level(t0, L + li, last_mm=False)
                    fill_dense_level(t0, 2 * li)
                    fill_dense_level(t0, 2 * li + 1,
                                     last=(li == depth - L - 1))
                    deep_level(t1, L + li, last_mm=False)
                    fill_dense_level(t1, 2 * li)
                    fill_dense_level(t1, 2 * li + 1,
                                     last=(li == depth - L - 1))
                emit_finish(t0)
                emit_finish(t1)

    return nc


_NC_CACHE = {}


def _get_nc(repeat=1):
    key = ("nc", repeat)
    if key not in _NC_CACHE:
        nc = bass.Bass("TRN2", target_bir_lowering=False, debug=False,
                       num_devices=N_CORES, num_swdge_queues=2)
        build_kernel(nc, repeat=repeat)
        _insert_library_loads(nc)
        _split_multi_waits(nc)
        _NC_CACHE[key] = nc
    return _NC_CACHE[key]


def make_kv(keys_flat_f32, values_flat_bf16):
    kv = np.empty((NK, KVW), dtype=np.float32)
    kv[:, :D] = keys_flat_f32
    kv[:, D:] = values_flat_bf16.view(np.float32)
    return kv


def _prep_inputs(x, keys, values):
    x = np.ascontiguousarray(np.asarray(x, dtype=np.float32))
    keys = np.asarray(keys, dtype=np.float32)
    values = np.asarray(values, dtype=np.float32)
    keys_flat = np.ascontiguousarray(keys.reshape(NK, D))
    values_flat = np.ascontiguousarray(values.reshape(NK, D)).astype(
        ml_dtypes.bfloat16)
    kv = make_kv(keys_flat, values_flat)
    kvb = np.empty((NK, KVBW), dtype=ml_dtypes.bfloat16)
    kvb[:, :D] = keys_flat.astype(ml_dtypes.bfloat16)
    kvb[:, D:] = values_flat

    kTs, vsh, expand, _ = host_prep(keys, values)
    identf, identb, iota_row, foldrep, mask64, ckm = host_consts()

    in_maps = []
    T = BL // P
    DC = D // P
    for c in range(N_CORES):
        x_shard = x[c * BL:(c + 1) * BL]
        xT4 = np.ascontiguousarray(
            x_shard.reshape(T, P, DC, P).transpose(3, 0, 2, 1))
        in_maps.append({
            "x": x_shard,
            "xT4": xT4,
            "kTs": kTs,
            "vsh": vsh,
            "expand": expand,
            "kv": kv,
            "kvb": kvb,
            "identf": identf,
            "identb": identb,
            "iota_row": iota_row,
            "foldrep": foldrep,
            "mask64": mask64,
            "ckm": ckm,
        })
    return in_maps


def kernel(x, keys, values):
    nc = _get_nc()
    in_maps = _prep_inputs(x, keys, values)
    res = run_bass_kernel_spmd(nc, in_maps, list(range(N_CORES)))
    y = np.concatenate([res.results[c]["y"] for c in range(N_CORES)], axis=0)
    return y.astype(np.float32)
